# revision 5
# baseline (speedup 1.0000x reference)
"""Trainium2 Bass kernel for a custom transformer block.

Sharding: 8 cores = 4 batches x 2 sequence halves. Each core computes the
full block (LN1 -> QKV -> windowed attention -> LN2 -> MLP -> residual) for
its 1024 query tokens; the KV window (last 1024 tokens of its batch) is
recomputed on both cores of a batch pair to avoid any collectives.
"""
import sys
import os

if "/opt/trn_rl_repo" not in sys.path:
    sys.path.insert(0, "/opt/trn_rl_repo")

import numpy as np
import ml_dtypes

B, S, D = 4, 2048, 1024
N_HEAD = 16
D_HEAD = 64
WINDOW = 1024
D_FF = 4096
EPS = 1e-5
ISD = float(1.0 / np.sqrt(D))  # 1/32
MASKVAL = -80.0   # exp(-80) = 1.8e-35: exact on ACT LUT, keeps fully-masked
KEEPVAL = 3e38    # rows uniform like the reference's -1e10 + softmax
P = 128

_CACHE = {}


def _build_program():
    import concourse.bacc as bacc
    import concourse.mybir as mybir
    from concourse.tile import TileContext
    from concourse.masks import make_identity

    F32 = mybir.dt.float32
    F32R = mybir.dt.float32r
    BF16 = mybir.dt.bfloat16
    AF = mybir.ActivationFunctionType
    ALU = mybir.AluOpType
    AX = mybir.AxisListType

    nc = bacc.Bacc("TRN2", target_bir_lowering=False, debug=False,
                   num_devices=8)

    xin_d = nc.dram_tensor("xin", [2 * WINDOW, D], F32, kind="ExternalInput")
    maskT_d = nc.dram_tensor("maskT", [WINDOW, WINDOW], BF16,
                             kind="ExternalInput")
    wq_d = nc.dram_tensor("wq", [D, D], F32R, kind="ExternalInput")
    wkv_d = nc.dram_tensor("wkv", [D, 2 * D], F32R, kind="ExternalInput")
    w1_d = nc.dram_tensor("w1", [D, D_FF], F32R, kind="ExternalInput")
    w2_d = nc.dram_tensor("w2", [D_FF, D], F32R, kind="ExternalInput")
    bqs_d = nc.dram_tensor("bqs", [P, 8], F32, kind="ExternalInput")
    bkvk_d = nc.dram_tensor("bkvk", [P, 8], F32, kind="ExternalInput")
    bkvvb_d = nc.dram_tensor("bkvvb", [P, D], F32, kind="ExternalInput")
    g1b_d = nc.dram_tensor("g1b", [P, D], F32, kind="ExternalInput")
    b1lb_d = nc.dram_tensor("b1lb", [P, D], F32, kind="ExternalInput")
    g2b_d = nc.dram_tensor("g2b", [P, D], F32, kind="ExternalInput")
    b2lb_d = nc.dram_tensor("b2lb", [P, D], F32, kind="ExternalInput")
    b1s_d = nc.dram_tensor("b1s", [P, 32], F32, kind="ExternalInput")
    b2s_d = nc.dram_tensor("b2s", [P, 8], F32, kind="ExternalInput")
    y_d = nc.dram_tensor("y", [WINDOW, D], F32, kind="ExternalOutput")

    with TileContext(nc) as tc:
        cpool = tc.alloc_tile_pool(name="const", bufs=1, side="left")
        ident = cpool.tile([P, P], F32)
        make_identity(nc, ident[:])
        onesc = cpool.tile([P, 16], F32)
        nc.vector.memset(onesc[:], 1.0)
        bqs = cpool.tile([P, 8], F32)
        bkvk = cpool.tile([P, 8], F32)
        b1s = cpool.tile([P, 32], F32)
        b2s = cpool.tile([P, 8], F32)
        nc.sync.dma_start(bqs[:], bqs_d[:])
        nc.sync.dma_start(bkvk[:], bkvk_d[:])
        nc.sync.dma_start(b1s[:], b1s_d[:])
        nc.sync.dma_start(b2s[:], b2s_d[:])

        # ---------------- Phase B: LN1 + transpose to dim-major ------------
        zTp = tc.alloc_tile_pool(name="zT", bufs=1, side="left")
        zqT = zTp.tile([P, 8, WINDOW], F32R)
        zwT = zTp.tile([P, 8, WINDOW], F32R)
        lnc = tc.alloc_tile_pool(name="lnc", bufs=1, side="left")
        xz = tc.alloc_tile_pool(name="xz", bufs=3, side="left")
        psB = tc.alloc_tile_pool(name="psB", bufs=3, space="PSUM")

        g1 = lnc.tile([P, D], F32)
        b1l = lnc.tile([P, D], F32)
        nc.sync.dma_start(g1[:], g1b_d[:])
        nc.sync.dma_start(b1l[:], b1lb_d[:])

        for t in range(16):
            xt = xz.tile([P, D], F32, tag="x")
            nc.sync.dma_start(xt[:], xin_d[t * P:(t + 1) * P, :])
            musum = xz.tile([P, 1], F32, tag="musum")
            nc.vector.reduce_sum(musum[:], xt[:], axis=AX.X)
            mu = xz.tile([P, 1], F32, tag="mu")
            nc.vector.tensor_scalar_mul(mu[:], musum[:], 1.0 / D)
            z = xz.tile([P, D], F32, tag="z")
            vsum = xz.tile([P, 1], F32, tag="vsum")
            # var pass writes scratch into z (overwritten below); vsum is
            # the real output: sum((x - mu) * x) = D * var
            nc.vector.scalar_tensor_tensor(
                z[:], xt[:], mu[:], xt[:],
                op0=ALU.subtract, op1=ALU.mult, accum_out=vsum[:])
            veps = xz.tile([P, 1], F32, tag="veps")
            nc.vector.tensor_scalar(veps[:], vsum[:], 1.0 / D, EPS,
                                    op0=ALU.mult, op1=ALU.add)
            sdv = xz.tile([P, 1], F32, tag="sdv")
            nc.scalar.sqrt(sdv[:], veps[:])
            rstd = xz.tile([P, 1], F32, tag="rstd")
            nc.vector.reciprocal(rstd[:], sdv[:])
            nc.vector.scalar_tensor_tensor(
                z[:], xt[:], mu[:], g1[:], op0=ALU.subtract, op1=ALU.mult)
            nc.vector.tensor_scalar_mul(z[:], z[:], rstd[:])
            nc.vector.tensor_tensor(z[:], z[:], b1l[:], op=ALU.add)
            dst = zqT if t < 8 else zwT
            col = (t % 8) * P
            for c in range(8):
                tp = psB.tile([P, P], F32, tag="tpB")
                nc.tensor.transpose(tp[:], z[:, c * P:(c + 1) * P], ident[:])
                nc.scalar.copy(dst[:, c, col:col + P], tp[:])

        psB.release()
        xz.release()
        lnc.release()

        # ---------------- Phase C: Q/K/V projections ------------------------
        qkvp = tc.alloc_tile_pool(name="qkv", bufs=1, side="right")
        qT = qkvp.tile([P, 8, WINDOW], F32R)      # q/sqrt(D), dim-major
        kT = qkvp.tile([P, 8, WINDOW], F32R)      # k, dim-major
        V = qkvp.tile([P, 8, N_HEAD * 65], F32R)  # token-major + ones col

        wst = tc.alloc_tile_pool(name="wst", bufs=1, side="left")
        psC = tc.alloc_tile_pool(name="psC", bufs=4, space="PSUM")

        # Q: weights stationary -> qT dim-major, scaled by 1/32
        wqr = wst.tile([P, 8, D], F32R, tag="wkres")
        for kc in range(8):
            nc.sync.dma_start(wqr[:, kc, :], wq_d[kc * P:(kc + 1) * P, :])
        for co in range(8):
            for qh in range(2):
                pp = psC.tile([P, 512], F32, tag="proj")
                for kc in range(8):
                    nc.tensor.matmul(
                        pp[:], wqr[:, kc, co * P:(co + 1) * P],
                        zqT[:, kc, qh * 512:(qh + 1) * 512],
                        start=(kc == 0), stop=(kc == 7))
                nc.scalar.activation(
                    qT[:, co, qh * 512:(qh + 1) * 512], pp[:],
                    AF.Identity, bias=bqs[:, co:co + 1], scale=ISD)
        # K: weights stationary -> kT dim-major
        wkr = wst.tile([P, 8, D], F32R, tag="wkres")
        for kc in range(8):
            nc.sync.dma_start(wkr[:, kc, :], wkv_d[kc * P:(kc + 1) * P, 0:D])
        for co in range(8):
            for qh in range(2):
                pp = psC.tile([P, 512], F32, tag="proj")
                for kc in range(8):
                    nc.tensor.matmul(
                        pp[:], wkr[:, kc, co * P:(co + 1) * P],
                        zwT[:, kc, qh * 512:(qh + 1) * 512],
                        start=(kc == 0), stop=(kc == 7))
                nc.scalar.activation(
                    kT[:, co, qh * 512:(qh + 1) * 512], pp[:],
                    AF.Identity, bias=bkvk[:, co:co + 1], scale=1.0)
        # V: activations stationary -> token-major, bias added via bcast tile
        bkvvb = wst.tile([P, D], F32, tag="bkvvb")
        nc.sync.dma_start(bkvvb[:], bkvvb_d[:])
        wvr = wst.tile([P, 8, D], F32R, tag="wkres")
        for kc in range(8):
            nc.sync.dma_start(wvr[:, kc, :],
                              wkv_d[kc * P:(kc + 1) * P, D:2 * D])
        for tt in range(8):
            for vh in range(2):
                pp = psC.tile([P, 512], F32, tag="proj")
                for kc in range(8):
                    nc.tensor.matmul(
                        pp[:], zwT[:, kc, tt * P:(tt + 1) * P],
                        wvr[:, kc, vh * 512:(vh + 1) * 512],
                        start=(kc == 0), stop=(kc == 7))
                vdst = V[:, tt, :].rearrange("p (h n) -> p h n", n=65)[
                    :, vh * 8:(vh + 1) * 8, 0:64]
                nc.vector.scalar_tensor_tensor(
                    vdst, pp[:].rearrange("p (h n) -> p h n", n=64), 0.0,
                    bkvvb[:, vh * 512:(vh + 1) * 512].rearrange(
                        "p (h n) -> p h n", n=64),
                    op0=ALU.add, op1=ALU.add)
            nc.scalar.copy(
                V[:, tt, :].rearrange("p (h n) -> p h n", n=65)[:, :, 64:65],
                onesc[:].rearrange("p (h n) -> p h n", n=1))

        psC.release()
        wst.release()
        zTp.release()

        # ---------------- Phase D: attention --------------------------------
        attnp = tc.alloc_tile_pool(name="attn", bufs=1, side="left")
        attn = attnp.tile([P, 8, D], F32)          # normalized attn out
        asum = attnp.tile([P, 8, N_HEAD], F32)     # per-head row sums

        mkp = tc.alloc_tile_pool(name="mk", bufs=1, side="left")
        sbD = tc.alloc_tile_pool(name="sbD", bufs=2, side="left")
        ptp = tc.alloc_tile_pool(name="ptp", bufs=10, side="right")
        psDs = tc.alloc_tile_pool(name="psDs", bufs=2, space="PSUM")
        psDa = tc.alloc_tile_pool(name="psDa", bufs=2, space="PSUM")
        psDt = tc.alloc_tile_pool(name="psDt", bufs=2, space="PSUM")

        maskT = mkp.tile([P, 8, WINDOW], BF16)
        nc.sync.dma_start(maskT[:], maskT_d.rearrange("(c p) n -> p c n", p=P))
        for h in range(N_HEAD):
            po, ch = (h % 2) * 64, h // 2
            pts = []
            for kc in range(8):
                sps = psDs.tile([P, 1024], F32, tag="s")
                for qh in range(2):
                    nc.tensor.matmul(
                        sps[:, qh * 512:(qh + 1) * 512],
                        kT[po:po + 64, ch, kc * P:(kc + 1) * P],
                        qT[po:po + 64, ch, qh * 512:(qh + 1) * 512],
                        start=True, stop=True)
                ssb = sbD.tile([P, 1024], F32, tag="ssb")
                nc.vector.tensor_tensor(ssb[:], sps[:], maskT[:, kc, :],
                                        op=ALU.min)
                pt = ptp.tile([P, 1024], F32R, tag="pt")
                nc.scalar.activation(pt[:], ssb[:], AF.Exp)
                pts.append(pt)
            oa = sbD.tile([65, 1024], F32, tag="oa")
            for qh in range(2):
                avp = psDa.tile([65, 512], F32, tag="av")
                for kc in range(8):
                    nc.tensor.matmul(
                        avp[:], V[:, kc, h * 65:(h + 1) * 65],
                        pts[kc][:, qh * 512:(qh + 1) * 512],
                        start=(kc == 0), stop=(kc == 7))
                nc.scalar.copy(oa[:, qh * 512:(qh + 1) * 512], avp[:])
            for t in range(8):
                tp = psDt.tile([P, 65], F32, tag="tp65")
                nc.tensor.transpose(tp[:], oa[:, t * P:(t + 1) * P],
                                    ident[0:65, 0:65])
                rinv = sbD.tile([P, 1], F32, tag="rinv")
                nc.vector.reciprocal(rinv[:], tp[:, 64:65])
                nc.scalar.activation(
                    attn[:, t, h * 64:(h + 1) * 64], tp[:, 0:64],
                    AF.Copy, scale=rinv[:], accum_out=asum[:, t, h:h + 1])

        psDt.release()
        psDa.release()
        psDs.release()
        sbD.release()
        mkp.release()
        ptp.release()
        qkvp.release()

        # ---------------- Phase E: LN2 + transpose ---------------------------
        z2Tp = tc.alloc_tile_pool(name="z2T", bufs=1, side="right")
        z2T = z2Tp.tile([P, 8, WINDOW], F32R)
        lnc2 = tc.alloc_tile_pool(name="lnc2", bufs=1, side="left")
        xz2 = tc.alloc_tile_pool(name="xz2", bufs=3, side="left")
        psE = tc.alloc_tile_pool(name="psE", bufs=3, space="PSUM")

        g2 = lnc2.tile([P, D], F32)
        b2l = lnc2.tile([P, D], F32)
        nc.sync.dma_start(g2[:], g2b_d[:])
        nc.sync.dma_start(b2l[:], b2lb_d[:])

        for t in range(8):
            at = attn[:, t, :]
            musum = xz2.tile([P, 1], F32, tag="musum2")
            nc.vector.reduce_sum(musum[:], asum[:, t, :], axis=AX.X)
            mu = xz2.tile([P, 1], F32, tag="mu2")
            nc.vector.tensor_scalar_mul(mu[:], musum[:], 1.0 / D)
            z = xz2.tile([P, D], F32, tag="zE")
            vsum = xz2.tile([P, 1], F32, tag="vsum2")
            nc.vector.scalar_tensor_tensor(
                z[:], at, mu[:], at,
                op0=ALU.subtract, op1=ALU.mult, accum_out=vsum[:])
            veps = xz2.tile([P, 1], F32, tag="veps2")
            nc.vector.tensor_scalar(veps[:], vsum[:], 1.0 / D, EPS,
                                    op0=ALU.mult, op1=ALU.add)
            sdv = xz2.tile([P, 1], F32, tag="sdv2")
            nc.scalar.sqrt(sdv[:], veps[:])
            rstd = xz2.tile([P, 1], F32, tag="rstd2")
            nc.vector.reciprocal(rstd[:], sdv[:])
            nc.vector.scalar_tensor_tensor(
                z[:], at, mu[:], g2[:], op0=ALU.subtract, op1=ALU.mult)
            nc.vector.tensor_scalar_mul(z[:], z[:], rstd[:])
            nc.vector.tensor_tensor(z[:], z[:], b2l[:], op=ALU.add)
            for c in range(8):
                tp = psE.tile([P, P], F32, tag="tpE")
                nc.tensor.transpose(tp[:], z[:, c * P:(c + 1) * P], ident[:])
                nc.scalar.copy(z2T[:, c, t * P:(t + 1) * P], tp[:])

        psE.release()
        xz2.release()
        lnc2.release()
        attnp.release()

        # ---------------- Phase F: MLP ---------------------------------------
        h2p = tc.alloc_tile_pool(name="h2acc", bufs=1, side="left")
        h2acc = h2p.tile([P, 8, WINDOW], F32)
        wf1 = tc.alloc_tile_pool(name="wf1", bufs=2, side="right")
        wf2 = tc.alloc_tile_pool(name="wf2", bufs=1, side="right")
        h1p = tc.alloc_tile_pool(name="h1p", bufs=1, side="left")
        psF1 = tc.alloc_tile_pool(name="psF1", bufs=3, space="PSUM")
        psF2 = tc.alloc_tile_pool(name="psF2", bufs=3, space="PSUM")

        for sc in range(4):
            w1r = wf1.tile([P, 8, 1024], F32R, tag="w1r")
            for kc in range(8):
                nc.sync.dma_start(
                    w1r[:, kc, :],
                    w1_d[kc * P:(kc + 1) * P, sc * 1024:(sc + 1) * 1024])
            h1 = h1p.tile([P, 8, WINDOW], F32R, tag="h1")
            for ft in range(8):
                for qh in range(2):
                    hp = psF1.tile([P, 512], F32, tag="h1ps")
                    for kc in range(8):
                        nc.tensor.matmul(
                            hp[:], w1r[:, kc, ft * P:(ft + 1) * P],
                            z2T[:, kc, qh * 512:(qh + 1) * 512],
                            start=(kc == 0), stop=(kc == 7))
                    nc.scalar.activation(
                        h1[:, ft, qh * 512:(qh + 1) * 512], hp[:], AF.Silu,
                        bias=b1s[:, sc * 8 + ft:sc * 8 + ft + 1], scale=1.0)
            w2r = wf2.tile([P, 8, 1024], F32R, tag="w2r")
            for kc in range(8):
                nc.sync.dma_start(
                    w2r[:, kc, :],
                    w2_d[(sc * 8 + kc) * P:(sc * 8 + kc + 1) * P, :])
            for co in range(8):
                for qh in range(2):
                    hp2 = psF2.tile([P, 512], F32, tag="h2ps")
                    for kc in range(8):
                        nc.tensor.matmul(
                            hp2[:], w2r[:, kc, co * P:(co + 1) * P],
                            h1[:, kc, qh * 512:(qh + 1) * 512],
                            start=(kc == 0), stop=(kc == 7))
                    dstp = h2acc[:, co, qh * 512:(qh + 1) * 512]
                    if sc == 0:
                        nc.vector.tensor_copy(dstp, hp2[:])
                    elif sc < 3:
                        nc.vector.tensor_tensor(dstp, hp2[:], dstp,
                                                op=ALU.add)
                    else:
                        nc.vector.scalar_tensor_tensor(
                            dstp, hp2[:], b2s[:, co:co + 1], dstp,
                            op0=ALU.add, op1=ALU.add)

        psF2.release()
        psF1.release()
        h1p.release()
        wf2.release()
        wf1.release()
        z2Tp.release()

        # tail: transpose + residual + store
        tailp = tc.alloc_tile_pool(name="tail", bufs=3, side="left")
        psF3 = tc.alloc_tile_pool(name="psF3", bufs=3, space="PSUM")
        for t in range(8):
            xq = tailp.tile([P, D], F32, tag="xq")
            nc.sync.dma_start(xq[:], xin_d[t * P:(t + 1) * P, :])
            y = tailp.tile([P, D], F32, tag="y")
            for co in range(8):
                tp = psF3.tile([P, P], F32, tag="tpF")
                nc.tensor.transpose(tp[:], h2acc[:, co, t * P:(t + 1) * P],
                                    ident[:])
                nc.vector.tensor_tensor(y[:, co * P:(co + 1) * P], tp[:],
                                        xq[:, co * P:(co + 1) * P],
                                        op=ALU.add)
            nc.sync.dma_start(y_d[t * P:(t + 1) * P, :], y[:])
        psF3.release()
        tailp.release()
        h2p.release()
        cpool.release()

    nc.compile()
    return nc


def _prep_inputs(inputs):
    x = np.ascontiguousarray(np.asarray(inputs["x"], dtype=np.float32))
    kpm = np.asarray(inputs["key_pad_mask"]).astype(bool)
    wq = np.ascontiguousarray(np.asarray(inputs["wq"], dtype=np.float32))
    wkv = np.ascontiguousarray(np.asarray(inputs["wkv"], dtype=np.float32))
    w1 = np.ascontiguousarray(np.asarray(inputs["w1"], dtype=np.float32))
    w2 = np.ascontiguousarray(np.asarray(inputs["w2"], dtype=np.float32))
    bq = np.asarray(inputs["bq"], dtype=np.float32)
    bkv = np.asarray(inputs["bkv"], dtype=np.float32)
    b1 = np.asarray(inputs["b1"], dtype=np.float32)
    b2 = np.asarray(inputs["b2"], dtype=np.float32)
    ln1_g = np.asarray(inputs["ln1_g"], dtype=np.float32)
    ln1_b = np.asarray(inputs["ln1_b"], dtype=np.float32)
    ln2_g = np.asarray(inputs["ln2_g"], dtype=np.float32)
    ln2_b = np.asarray(inputs["ln2_b"], dtype=np.float32)

    shared = {
        "wq": wq,
        "wkv": wkv,
        "w1": w1,
        "w2": w2,
        "bqs": np.ascontiguousarray((bq * ISD).reshape(8, P).T),
        "bkvk": np.ascontiguousarray(bkv[0:D].reshape(8, P).T),
        "bkvvb": np.ascontiguousarray(
            np.broadcast_to(bkv[D:2 * D], (P, D)).astype(np.float32)),
        "g1b": np.ascontiguousarray(np.broadcast_to(ln1_g, (P, D))),
        "b1lb": np.ascontiguousarray(np.broadcast_to(ln1_b, (P, D))),
        "g2b": np.ascontiguousarray(np.broadcast_to(ln2_g, (P, D))),
        "b2lb": np.ascontiguousarray(np.broadcast_to(ln2_b, (P, D))),
        "b1s": np.ascontiguousarray(b1.reshape(32, P).T),
        "b2s": np.ascontiguousarray(b2.reshape(8, P).T),
    }

    j = np.arange(WINDOW)[:, None]   # key index within window (row)
    i = np.arange(WINDOW)[None, :]   # local query index (col)
    in_maps = []
    for core in range(8):
        b, h = core // 2, core % 2
        xq = x[b, h * WINDOW:(h + 1) * WINDOW]
        xw = x[b, S - WINDOW:S]
        masked = (j > h * WINDOW + i) | kpm[b, S - WINDOW:S][:, None]
        maskT = np.where(masked, np.float32(MASKVAL),
                         np.float32(KEEPVAL)).astype(ml_dtypes.bfloat16)
        m = dict(shared)
        m["xin"] = np.ascontiguousarray(np.concatenate([xq, xw], axis=0))
        m["maskT"] = np.ascontiguousarray(maskT)
        in_maps.append(m)
    return in_maps


def kernel(**inputs):
    from concourse.bass_utils import run_bass_kernel_spmd

    if "nc" not in _CACHE:
        _CACHE["nc"] = _build_program()
    nc = _CACHE["nc"]

    in_maps = _prep_inputs(inputs)
    trace = os.environ.get("KERNEL_TRACE", "0") == "1"
    res = run_bass_kernel_spmd(nc, in_maps, core_ids=list(range(8)),
                               trace=trace)
    if res.exec_time_ns is not None:
        print(f"HW exec time: {res.exec_time_ns} ns")
        _CACHE["exec_time_ns"] = res.exec_time_ns
    out = np.empty((B, S, D), dtype=np.float32)
    for core in range(8):
        b, h = core // 2, core % 2
        out[b, h * WINDOW:(h + 1) * WINDOW] = res.results[core]["y"]
    return out


# revision 13
# speedup vs baseline: 5.0915x; 5.0915x over previous
"""Trainium2 Bass kernel for a custom transformer block.

Sharding: 8 cores = 4 batches x 2 sequence halves. Each core computes the
full block (LN1 -> QKV -> windowed attention -> LN2 -> MLP -> residual) for
its 1024 query tokens; the KV window (last 1024 tokens of its batch) is
recomputed on both cores of a batch pair to avoid any collectives.

Layout strategy: layernorms run token-major (free-dim reductions), matmul
operands are kept dim-major via PE transposes whose PSUM evacuation also
applies the LN gain/bias (per-partition scale/bias on the Scalar engine).
Attention scores are computed transposed ([key, query]) so no softmax
transposes are needed; row sums come free from an extra ones-column on V
and the normalization folds into the attention-output evacuation. All
matmuls run as float32r.
"""
import sys
import os

if "/opt/trn_rl_repo" not in sys.path:
    sys.path.insert(0, "/opt/trn_rl_repo")

import numpy as np
import ml_dtypes

B, S, D = 4, 2048, 1024
N_HEAD = 16
D_HEAD = 64
WINDOW = 1024
D_FF = 4096
EPS = 1e-5
ISD = float(1.0 / np.sqrt(D))  # 1/32
MASKVAL = -80.0   # exp(-80) = 1.8e-35: exact on ACT LUT, keeps fully-masked
KEEPVAL = 3e38    # rows uniform like the reference's -1e10 + softmax
P = 128

_CACHE = {}


def _build_program():
    import concourse.bacc as bacc
    import concourse.mybir as mybir
    from concourse.tile import TileContext
    from concourse.masks import make_identity

    F32 = mybir.dt.float32
    F32R = mybir.dt.float32r
    BF16 = mybir.dt.bfloat16
    AF = mybir.ActivationFunctionType
    ALU = mybir.AluOpType
    AX = mybir.AxisListType

    nc = bacc.Bacc("TRN2", target_bir_lowering=False, debug=False,
                   num_devices=8)

    xin_d = nc.dram_tensor("xin", [2 * WINDOW, D], F32, kind="ExternalInput")
    maskT_d = nc.dram_tensor("maskT", [WINDOW, WINDOW], BF16,
                             kind="ExternalInput")
    wq_d = nc.dram_tensor("wq", [D, D], F32R, kind="ExternalInput")
    wkv_d = nc.dram_tensor("wkv", [D, 2 * D], F32R, kind="ExternalInput")
    w1_d = nc.dram_tensor("w1", [D, D_FF], F32R, kind="ExternalInput")
    w2_d = nc.dram_tensor("w2", [D_FF, D], F32R, kind="ExternalInput")
    bqs_d = nc.dram_tensor("bqs", [P, 8], F32, kind="ExternalInput")
    bkvk_d = nc.dram_tensor("bkvk", [P, 8], F32, kind="ExternalInput")
    bkvvb_d = nc.dram_tensor("bkvvb", [P, D], F32, kind="ExternalInput")
    g1dm_d = nc.dram_tensor("g1dm", [P, 8], F32, kind="ExternalInput")
    b1dm_d = nc.dram_tensor("b1dm", [P, 8], F32, kind="ExternalInput")
    g2dm_d = nc.dram_tensor("g2dm", [P, 8], F32, kind="ExternalInput")
    b2dm_d = nc.dram_tensor("b2dm", [P, 8], F32, kind="ExternalInput")
    b1s_d = nc.dram_tensor("b1s", [P, 32], F32, kind="ExternalInput")
    b2s_d = nc.dram_tensor("b2s", [P, 8], F32, kind="ExternalInput")
    xinT_d = nc.dram_tensor("xinT", [D, WINDOW], F32, kind="ExternalInput")
    y_d = nc.dram_tensor("y", [D, WINDOW], F32, kind="ExternalOutput")

    with TileContext(nc) as tc:
        cpool = tc.alloc_tile_pool(name="const", bufs=1, side="left")
        ident = cpool.tile([P, P], F32)
        make_identity(nc, ident[:])
        smallc = cpool.tile([P, 104], F32)
        bqs = smallc[:, 0:8]
        bkvk = smallc[:, 8:16]
        b1s = smallc[:, 16:48]
        b2s = smallc[:, 48:56]
        onesc = smallc[:, 56:72]
        g1dm = smallc[:, 72:80]
        b1dm = smallc[:, 80:88]
        g2dm = smallc[:, 88:96]
        b2dm = smallc[:, 96:104]
        nc.vector.memset(onesc, 1.0)
        nc.sync.dma_start(bqs, bqs_d[:])
        nc.sync.dma_start(bkvk, bkvk_d[:])
        nc.sync.dma_start(b1s, b1s_d[:])
        nc.sync.dma_start(b2s, b2s_d[:])
        nc.sync.dma_start(g1dm, g1dm_d[:])
        nc.sync.dma_start(b1dm, b1dm_d[:])
        nc.sync.dma_start(g2dm, g2dm_d[:])
        nc.sync.dma_start(b2dm, b2dm_d[:])

        # ---------------- Phase B: LN1 + transpose to dim-major ------------
        # z = (x - mu) * rstd in token-major; gain/bias applied per-dim
        # during the transposed PSUM evacuation on ScalarE.
        zTp = tc.alloc_tile_pool(name="zT", bufs=1, side="left")
        zqT = zTp.tile([P, 8, WINDOW], F32R)
        zwT = zTp.tile([P, 8, WINDOW], F32R)
        xz = tc.alloc_tile_pool(name="xz", bufs=3, side="left")
        psB = tc.alloc_tile_pool(name="psB", bufs=3, space="PSUM")

        for t in range(16):
            xt = xz.tile([P, D], F32, tag="x")
            nc.sync.dma_start(xt[:], xin_d[t * P:(t + 1) * P, :])
            st = xz.tile([P, 8], F32, tag="stats")
            musum, mu, vsum = st[:, 0:1], st[:, 1:2], st[:, 2:3]
            veps, sdv, rstd = st[:, 4:5], st[:, 5:6], st[:, 6:7]
            nc.vector.reduce_sum(musum, xt[:], axis=AX.X)
            nc.vector.tensor_scalar_mul(mu, musum, 1.0 / D)
            z = xz.tile([P, D], F32, tag="z")
            # scratch into z; vsum = sum((x - mu) * x) = D * var
            nc.vector.scalar_tensor_tensor(
                z[:], xt[:], mu, xt[:],
                op0=ALU.subtract, op1=ALU.mult, accum_out=vsum)
            nc.vector.tensor_scalar(veps, vsum, 1.0 / D, EPS,
                                    op0=ALU.mult, op1=ALU.add)
            nc.scalar.sqrt(sdv, veps)
            nc.vector.reciprocal(rstd, sdv)
            nc.vector.tensor_scalar(z[:], xt[:], mu, rstd,
                                    op0=ALU.subtract, op1=ALU.mult)
            dst = zqT if t < 8 else zwT
            col = (t % 8) * P
            for c in range(8):
                tp = psB.tile([P, P], F32, tag="tpB")
                nc.tensor.transpose(tp[:], z[:, c * P:(c + 1) * P], ident[:])
                nc.scalar.activation(dst[:, c, col:col + P], tp[:],
                                     AF.Identity, bias=b1dm[:, c:c + 1],
                                     scale=g1dm[:, c:c + 1])

        psB.release()
        xz.release()

        # ---------------- Phase C: Q/K/V projections ------------------------
        qkvp = tc.alloc_tile_pool(name="qkv", bufs=1, side="right")
        qT = qkvp.tile([P, 8, WINDOW], F32R)      # q/sqrt(D), dim-major
        kT = qkvp.tile([P, 8, WINDOW], F32R)      # k, dim-major
        V = qkvp.tile([P, 8, N_HEAD * 65], F32R)  # token-major + ones col

        wst = tc.alloc_tile_pool(name="wst", bufs=1, side="left")
        psC = tc.alloc_tile_pool(name="psC", bufs=4, space="PSUM")

        # Q: weights stationary -> qT dim-major, scaled by 1/32
        wqr = wst.tile([P, 8, D], F32R, tag="wkres")
        for kc in range(8):
            nc.sync.dma_start(wqr[:, kc, :], wq_d[kc * P:(kc + 1) * P, :])
        for co in range(8):
            for qh in range(2):
                pp = psC.tile([P, 512], F32, tag="proj")
                for kc in range(8):
                    nc.tensor.matmul(
                        pp[:], wqr[:, kc, co * P:(co + 1) * P],
                        zqT[:, kc, qh * 512:(qh + 1) * 512],
                        start=(kc == 0), stop=(kc == 7))
                nc.scalar.activation(
                    qT[:, co, qh * 512:(qh + 1) * 512], pp[:],
                    AF.Identity, bias=bqs[:, co:co + 1], scale=ISD)
        # K: weights stationary -> kT dim-major
        wkr = wst.tile([P, 8, D], F32R, tag="wkres")
        for kc in range(8):
            nc.sync.dma_start(wkr[:, kc, :], wkv_d[kc * P:(kc + 1) * P, 0:D])
        for co in range(8):
            for qh in range(2):
                pp = psC.tile([P, 512], F32, tag="proj")
                for kc in range(8):
                    nc.tensor.matmul(
                        pp[:], wkr[:, kc, co * P:(co + 1) * P],
                        zwT[:, kc, qh * 512:(qh + 1) * 512],
                        start=(kc == 0), stop=(kc == 7))
                nc.scalar.activation(
                    kT[:, co, qh * 512:(qh + 1) * 512], pp[:],
                    AF.Identity, bias=bkvk[:, co:co + 1], scale=1.0)
        # V: activations stationary -> token-major, bias added via bcast tile
        bkvvb = wst.tile([P, D], F32, tag="bkvvb")
        nc.sync.dma_start(bkvvb[:], bkvvb_d[:])
        wvr = wst.tile([P, 8, D], F32R, tag="wkres")
        for kc in range(8):
            nc.sync.dma_start(wvr[:, kc, :],
                              wkv_d[kc * P:(kc + 1) * P, D:2 * D])
        for tt in range(8):
            for vh in range(2):
                pp = psC.tile([P, 512], F32, tag="proj")
                for kc in range(8):
                    nc.tensor.matmul(
                        pp[:], zwT[:, kc, tt * P:(tt + 1) * P],
                        wvr[:, kc, vh * 512:(vh + 1) * 512],
                        start=(kc == 0), stop=(kc == 7))
                vdst = V[:, tt, :].rearrange("p (h n) -> p h n", n=65)[
                    :, vh * 8:(vh + 1) * 8, 0:64]
                nc.vector.scalar_tensor_tensor(
                    vdst, pp[:].rearrange("p (h n) -> p h n", n=64), 0.0,
                    bkvvb[:, vh * 512:(vh + 1) * 512].rearrange(
                        "p (h n) -> p h n", n=64),
                    op0=ALU.add, op1=ALU.add)
            nc.scalar.copy(
                V[:, tt, :].rearrange("p (h n) -> p h n", n=65)[:, :, 64:65],
                onesc.rearrange("p (h n) -> p h n", n=1))

        psC.release()
        wst.release()
        zTp.release()

        # ---------------- Phase D: attention --------------------------------
        attnp = tc.alloc_tile_pool(name="attn", bufs=1, side="left")
        attn = attnp.tile([P, 8, D], F32)          # normalized attn out
        asum = attnp.tile([P, 8, N_HEAD], F32)     # per-head row sums
        rinva = attnp.tile([P, 8, N_HEAD], F32)    # per-head 1/rowsum

        mkp = tc.alloc_tile_pool(name="mk", bufs=1, side="left")
        sbD = tc.alloc_tile_pool(name="sbD", bufs=2, side="left")
        ssbp = tc.alloc_tile_pool(name="ssbp", bufs=4, side="left")
        ptp = tc.alloc_tile_pool(name="ptp", bufs=9, side="right")
        psDs = tc.alloc_tile_pool(name="psDs", bufs=5, space="PSUM")
        psDa = tc.alloc_tile_pool(name="psDa", bufs=2, space="PSUM")
        psDt = tc.alloc_tile_pool(name="psDt", bufs=1, space="PSUM")

        maskT = mkp.tile([P, 8, WINDOW], BF16)
        nc.sync.dma_start(maskT[:], maskT_d.rearrange("(c p) n -> p c n", p=P))
        for hp in range(N_HEAD // 2):
            pair = (2 * hp, 2 * hp + 1)
            # scores + mask + exp; consecutive matmuls alternate PE row
            # groups (partitions 0-63 vs 64-127) so LDWEIGHTS pulls ahead
            pts = {h: [] for h in pair}
            for kc in range(8):
                ssbs = {}
                for h in pair:
                    ptile = ptp.tile([P, 1024], F32R, tag="pt")
                    pts[h].append(ptile)
                    stile = ssbp.tile([P, 1024], F32, tag="ssb")
                    ssbs[h] = stile
                for qh in range(2):
                    for h in pair:
                        po, ch = (h % 2) * 64, h // 2
                        sps = psDs.tile([P, 512], F32, tag="s")
                        nc.tensor.matmul(
                            sps[:],
                            kT[po:po + 64, ch, kc * P:(kc + 1) * P],
                            qT[po:po + 64, ch, qh * 512:(qh + 1) * 512],
                            start=True, stop=True)
                        nc.vector.tensor_tensor(
                            ssbs[h][:, qh * 512:(qh + 1) * 512], sps[:],
                            maskT[:, kc, qh * 512:(qh + 1) * 512], op=ALU.min)
                for h in pair:
                    nc.scalar.activation(pts[h][kc][:], ssbs[h][:], AF.Exp)
            for h in pair:
                oa = sbD.tile([65, 1024], F32, tag="oa")
                for qh in range(2):
                    avp = psDa.tile([65, 512], F32, tag="av")
                    for kc in range(8):
                        nc.tensor.matmul(
                            avp[:], V[:, kc, h * 65:(h + 1) * 65],
                            pts[h][kc][:, qh * 512:(qh + 1) * 512],
                            start=(kc == 0), stop=(kc == 7))
                    nc.scalar.copy(oa[:, qh * 512:(qh + 1) * 512], avp[:])
                for t in range(8):
                    tp = psDt.tile([P, 65], F32, tag="tp65")
                    nc.tensor.transpose(tp[:], oa[:, t * P:(t + 1) * P],
                                        ident[0:65, 0:65])
                    rinv = rinva[:, t, h:h + 1]
                    nc.vector.reciprocal(rinv, tp[:, 64:65])
                    nc.scalar.activation(
                        attn[:, t, h * 64:(h + 1) * 64], tp[:, 0:64],
                        AF.Copy, scale=rinv, accum_out=asum[:, t, h:h + 1])

        psDt.release()
        psDa.release()
        psDs.release()
        ptp.release()
        ssbp.release()
        sbD.release()
        mkp.release()
        qkvp.release()

        # ---------------- Phase E: LN2 + transpose ---------------------------
        z2Tp = tc.alloc_tile_pool(name="z2T", bufs=1, side="right")
        z2T = z2Tp.tile([P, 8, WINDOW], F32R)
        xz2 = tc.alloc_tile_pool(name="xz2", bufs=3, side="left")
        psE = tc.alloc_tile_pool(name="psE", bufs=3, space="PSUM")

        for t in range(8):
            at = attn[:, t, :]
            st = xz2.tile([P, 8], F32, tag="stats2")
            musum, mu, vsum = st[:, 0:1], st[:, 1:2], st[:, 2:3]
            veps, sdv, rstd = st[:, 4:5], st[:, 5:6], st[:, 6:7]
            nc.vector.reduce_sum(musum, asum[:, t, :], axis=AX.X)
            nc.vector.tensor_scalar_mul(mu, musum, 1.0 / D)
            z = xz2.tile([P, D], F32, tag="zE")
            nc.vector.scalar_tensor_tensor(
                z[:], at, mu, at,
                op0=ALU.subtract, op1=ALU.mult, accum_out=vsum)
            nc.vector.tensor_scalar(veps, vsum, 1.0 / D, EPS,
                                    op0=ALU.mult, op1=ALU.add)
            nc.scalar.sqrt(sdv, veps)
            nc.vector.reciprocal(rstd, sdv)
            nc.vector.tensor_scalar(z[:], at, mu, rstd,
                                    op0=ALU.subtract, op1=ALU.mult)
            for c in range(8):
                tp = psE.tile([P, P], F32, tag="tpE")
                nc.tensor.transpose(tp[:], z[:, c * P:(c + 1) * P], ident[:])
                nc.scalar.activation(z2T[:, c, t * P:(t + 1) * P], tp[:],
                                     AF.Identity, bias=b2dm[:, c:c + 1],
                                     scale=g2dm[:, c:c + 1])

        psE.release()
        xz2.release()
        attnp.release()

        # ---------------- Phase F: MLP ---------------------------------------
        h2p = tc.alloc_tile_pool(name="h2acc", bufs=1, side="left")
        h2acc = h2p.tile([P, 8, WINDOW], F32)
        wf1 = tc.alloc_tile_pool(name="wf1", bufs=2, side="right")
        wf2 = tc.alloc_tile_pool(name="wf2", bufs=1, side="right")
        h1p = tc.alloc_tile_pool(name="h1p", bufs=1, side="left")
        psF1 = tc.alloc_tile_pool(name="psF1", bufs=3, space="PSUM")
        psF2 = tc.alloc_tile_pool(name="psF2", bufs=3, space="PSUM")

        for sc in range(4):
            w1r = wf1.tile([P, 8, 1024], F32R, tag="w1r")
            for kc in range(8):
                nc.sync.dma_start(
                    w1r[:, kc, :],
                    w1_d[kc * P:(kc + 1) * P, sc * 1024:(sc + 1) * 1024])
            h1 = h1p.tile([P, 8, WINDOW], F32R, tag="h1")
            for ft in range(8):
                for qh in range(2):
                    hp = psF1.tile([P, 512], F32, tag="h1ps")
                    for kc in range(8):
                        nc.tensor.matmul(
                            hp[:], w1r[:, kc, ft * P:(ft + 1) * P],
                            z2T[:, kc, qh * 512:(qh + 1) * 512],
                            start=(kc == 0), stop=(kc == 7))
                    nc.scalar.activation(
                        h1[:, ft, qh * 512:(qh + 1) * 512], hp[:], AF.Silu,
                        bias=b1s[:, sc * 8 + ft:sc * 8 + ft + 1], scale=1.0)
            w2r = wf2.tile([P, 8, 1024], F32R, tag="w2r")
            for kc in range(8):
                nc.sync.dma_start(
                    w2r[:, kc, :],
                    w2_d[(sc * 8 + kc) * P:(sc * 8 + kc + 1) * P, :])
            for co in range(8):
                for qh in range(2):
                    hp2 = psF2.tile([P, 512], F32, tag="h2ps")
                    for kc in range(8):
                        nc.tensor.matmul(
                            hp2[:], w2r[:, kc, co * P:(co + 1) * P],
                            h1[:, kc, qh * 512:(qh + 1) * 512],
                            start=(kc == 0), stop=(kc == 7))
                    dstp = h2acc[:, co, qh * 512:(qh + 1) * 512]
                    if sc == 0:
                        nc.vector.tensor_copy(dstp, hp2[:])
                    elif sc < 3:
                        nc.vector.tensor_tensor(dstp, hp2[:], dstp,
                                                op=ALU.add)
                    else:
                        nc.vector.scalar_tensor_tensor(
                            dstp, hp2[:], b2s[:, co:co + 1], dstp,
                            op0=ALU.add, op1=ALU.add)

        psF2.release()
        psF1.release()
        h1p.release()
        wf2.release()
        wf1.release()
        z2Tp.release()

        # tail: residual add in dim-major (host supplies x^T and
        # transposes y back), no PE transposes needed
        tailp = tc.alloc_tile_pool(name="tail", bufs=3, side="left")
        xinTp = tc.alloc_tile_pool(name="xinT", bufs=1, side="left")
        xinT = xinTp.tile([P, 8, WINDOW], F32)
        nc.sync.dma_start(xinT[:], xinT_d.rearrange("(c p) n -> p c n", p=P))
        for co in range(8):
            y = tailp.tile([P, WINDOW], F32, tag="y")
            nc.vector.tensor_tensor(y[:], h2acc[:, co, :], xinT[:, co, :],
                                    op=ALU.add)
            nc.sync.dma_start(y_d[co * P:(co + 1) * P, :], y[:])
        xinTp.release()
        tailp.release()
        h2p.release()
        cpool.release()

    nc.compile()
    return nc


def _prep_inputs(inputs):
    x = np.ascontiguousarray(np.asarray(inputs["x"], dtype=np.float32))
    kpm = np.asarray(inputs["key_pad_mask"]).astype(bool)
    wq = np.ascontiguousarray(np.asarray(inputs["wq"], dtype=np.float32))
    wkv = np.ascontiguousarray(np.asarray(inputs["wkv"], dtype=np.float32))
    w1 = np.ascontiguousarray(np.asarray(inputs["w1"], dtype=np.float32))
    w2 = np.ascontiguousarray(np.asarray(inputs["w2"], dtype=np.float32))
    bq = np.asarray(inputs["bq"], dtype=np.float32)
    bkv = np.asarray(inputs["bkv"], dtype=np.float32)
    b1 = np.asarray(inputs["b1"], dtype=np.float32)
    b2 = np.asarray(inputs["b2"], dtype=np.float32)
    ln1_g = np.asarray(inputs["ln1_g"], dtype=np.float32)
    ln1_b = np.asarray(inputs["ln1_b"], dtype=np.float32)
    ln2_g = np.asarray(inputs["ln2_g"], dtype=np.float32)
    ln2_b = np.asarray(inputs["ln2_b"], dtype=np.float32)

    def dm(v):  # [D] -> [P, 8] dim-major chunk layout
        return np.ascontiguousarray(v.reshape(8, P).T)

    shared = {
        "wq": wq,
        "wkv": wkv,
        "w1": w1,
        "w2": w2,
        "bqs": np.ascontiguousarray((bq * ISD).reshape(8, P).T),
        "bkvk": dm(bkv[0:D]),
        "bkvvb": np.ascontiguousarray(
            np.broadcast_to(bkv[D:2 * D], (P, D)).astype(np.float32)),
        "g1dm": dm(ln1_g),
        "b1dm": dm(ln1_b),
        "g2dm": dm(ln2_g),
        "b2dm": dm(ln2_b),
        "b1s": np.ascontiguousarray(b1.reshape(32, P).T),
        "b2s": dm(b2),
    }

    j = np.arange(WINDOW)[:, None]   # key index within window (row)
    i = np.arange(WINDOW)[None, :]   # local query index (col)
    in_maps = []
    for core in range(8):
        b, h = core // 2, core % 2
        xq = x[b, h * WINDOW:(h + 1) * WINDOW]
        xw = x[b, S - WINDOW:S]
        masked = (j > h * WINDOW + i) | kpm[b, S - WINDOW:S][:, None]
        maskT = np.where(masked, np.float32(MASKVAL),
                         np.float32(KEEPVAL)).astype(ml_dtypes.bfloat16)
        m = dict(shared)
        m["xin"] = np.ascontiguousarray(np.concatenate([xq, xw], axis=0))
        m["xinT"] = np.ascontiguousarray(xq.T)
        m["maskT"] = np.ascontiguousarray(maskT)
        in_maps.append(m)
    return in_maps


def kernel(**inputs):
    from concourse.bass_utils import run_bass_kernel_spmd

    if "nc" not in _CACHE:
        _CACHE["nc"] = _build_program()
    nc = _CACHE["nc"]

    in_maps = _prep_inputs(inputs)
    trace = os.environ.get("KERNEL_TRACE", "0") == "1"
    res = run_bass_kernel_spmd(nc, in_maps, core_ids=list(range(8)),
                               trace=trace)
    if res.exec_time_ns is not None:
        print(f"HW exec time: {res.exec_time_ns} ns")
        _CACHE["exec_time_ns"] = res.exec_time_ns
    out = np.empty((B, S, D), dtype=np.float32)
    for core in range(8):
        b, h = core // 2, core % 2
        out[b, h * WINDOW:(h + 1) * WINDOW] = res.results[core]["y"].T
    return out


# revision 15
# speedup vs baseline: 5.3132x; 1.0435x over previous
"""Trainium2 Bass kernel for a custom transformer block.

Sharding: 8 cores = 4 batches x 2 sequence halves. Each core computes the
full block (LN1 -> QKV -> windowed attention -> LN2 -> MLP -> residual) for
its 1024 query tokens; the KV window (last 1024 tokens of its batch) is
recomputed on both cores of a batch pair to avoid any collectives.

Layout strategy: layernorms run token-major (free-dim reductions), matmul
operands are kept dim-major via PE transposes whose PSUM evacuation also
applies the LN gain/bias (per-partition scale/bias on the Scalar engine).
Attention scores are computed transposed ([key, query]) so no softmax
transposes are needed; row sums come free from an extra ones-column on V
and the normalization folds into the attention-output evacuation. All
matmuls run as float32r.
"""
import sys
import os

if "/opt/trn_rl_repo" not in sys.path:
    sys.path.insert(0, "/opt/trn_rl_repo")

import numpy as np
import ml_dtypes

B, S, D = 4, 2048, 1024
N_HEAD = 16
D_HEAD = 64
WINDOW = 1024
D_FF = 4096
EPS = 1e-5
ISD = float(1.0 / np.sqrt(D))  # 1/32
MASKVAL = -80.0   # exp(-80) = 1.8e-35: exact on ACT LUT, keeps fully-masked
KEEPVAL = 3e38    # rows uniform like the reference's -1e10 + softmax
P = 128

_CACHE = {}


def _build_program():
    import concourse.bacc as bacc
    import concourse.mybir as mybir
    from concourse.tile import TileContext
    from concourse.masks import make_identity

    F32 = mybir.dt.float32
    F32R = mybir.dt.float32r
    BF16 = mybir.dt.bfloat16
    AF = mybir.ActivationFunctionType
    ALU = mybir.AluOpType
    AX = mybir.AxisListType

    nc = bacc.Bacc("TRN2", target_bir_lowering=False, debug=False,
                   num_devices=8)

    xin_d = nc.dram_tensor("xin", [2 * WINDOW, D], F32, kind="ExternalInput")
    maskT_d = nc.dram_tensor("maskT", [WINDOW, WINDOW], BF16,
                             kind="ExternalInput")
    wq_d = nc.dram_tensor("wq", [D, D], F32R, kind="ExternalInput")
    wkv_d = nc.dram_tensor("wkv", [D, 2 * D], F32R, kind="ExternalInput")
    w1_d = nc.dram_tensor("w1", [D, D_FF], F32R, kind="ExternalInput")
    w2_d = nc.dram_tensor("w2", [D_FF, D], F32R, kind="ExternalInput")
    bqs_d = nc.dram_tensor("bqs", [P, 8], F32, kind="ExternalInput")
    bkvk_d = nc.dram_tensor("bkvk", [P, 8], F32, kind="ExternalInput")
    bkvvb_d = nc.dram_tensor("bkvvb", [P, D], F32, kind="ExternalInput")
    g1dm_d = nc.dram_tensor("g1dm", [P, 8], F32, kind="ExternalInput")
    b1dm_d = nc.dram_tensor("b1dm", [P, 8], F32, kind="ExternalInput")
    g2dm_d = nc.dram_tensor("g2dm", [P, 8], F32, kind="ExternalInput")
    b2dm_d = nc.dram_tensor("b2dm", [P, 8], F32, kind="ExternalInput")
    b1s_d = nc.dram_tensor("b1s", [P, 32], F32, kind="ExternalInput")
    b2s_d = nc.dram_tensor("b2s", [P, 8], F32, kind="ExternalInput")
    xinT_d = nc.dram_tensor("xinT", [D, WINDOW], F32, kind="ExternalInput")
    y_d = nc.dram_tensor("y", [D, WINDOW], F32, kind="ExternalOutput")

    with TileContext(nc) as tc:
        cpool = tc.alloc_tile_pool(name="const", bufs=1, side="left")
        ident = cpool.tile([P, P], F32)
        make_identity(nc, ident[:])
        smallc = cpool.tile([P, 104], F32)
        bqs = smallc[:, 0:8]
        bkvk = smallc[:, 8:16]
        b1s = smallc[:, 16:48]
        b2s = smallc[:, 48:56]
        onesc = smallc[:, 56:72]
        g1dm = smallc[:, 72:80]
        b1dm = smallc[:, 80:88]
        g2dm = smallc[:, 88:96]
        b2dm = smallc[:, 96:104]
        nc.vector.memset(onesc, 1.0)
        nc.sync.dma_start(bqs, bqs_d[:])
        nc.sync.dma_start(bkvk, bkvk_d[:])
        nc.sync.dma_start(b1s, b1s_d[:])
        nc.sync.dma_start(b2s, b2s_d[:])
        nc.sync.dma_start(g1dm, g1dm_d[:])
        nc.sync.dma_start(b1dm, b1dm_d[:])
        nc.sync.dma_start(g2dm, g2dm_d[:])
        nc.sync.dma_start(b2dm, b2dm_d[:])

        # ---------------- Phase B: LN1 + transpose to dim-major ------------
        # z = (x - mu) * rstd in token-major; gain/bias applied per-dim
        # during the transposed PSUM evacuation on ScalarE.
        zTp = tc.alloc_tile_pool(name="zT", bufs=1, side="left")
        zqT = zTp.tile([P, 8, WINDOW], F32R)
        zwT = zTp.tile([P, 8, WINDOW], F32R)
        xz = tc.alloc_tile_pool(name="xz", bufs=3, side="left")
        psB = tc.alloc_tile_pool(name="psB", bufs=3, space="PSUM")

        def ln1_tile(t):
            xt = xz.tile([P, D], F32, tag="x")
            nc.sync.dma_start(xt[:], xin_d[t * P:(t + 1) * P, :])
            st = xz.tile([P, 8], F32, tag="stats")
            musum, mu, vsum = st[:, 0:1], st[:, 1:2], st[:, 2:3]
            veps, sdv, rstd = st[:, 4:5], st[:, 5:6], st[:, 6:7]
            nc.vector.reduce_sum(musum, xt[:], axis=AX.X)
            nc.vector.tensor_scalar_mul(mu, musum, 1.0 / D)
            z = xz.tile([P, D], F32, tag="z")
            # scratch into z; vsum = sum((x - mu) * x) = D * var
            nc.vector.scalar_tensor_tensor(
                z[:], xt[:], mu, xt[:],
                op0=ALU.subtract, op1=ALU.mult, accum_out=vsum)
            nc.vector.tensor_scalar(veps, vsum, 1.0 / D, EPS,
                                    op0=ALU.mult, op1=ALU.add)
            nc.scalar.sqrt(sdv, veps)
            nc.vector.reciprocal(rstd, sdv)
            nc.vector.tensor_scalar(z[:], xt[:], mu, rstd,
                                    op0=ALU.subtract, op1=ALU.mult)
            dst = zqT if t < 8 else zwT
            col = (t % 8) * P
            for c in range(8):
                tp = psB.tile([P, P], F32, tag="tpB")
                nc.tensor.transpose(tp[:], z[:, c * P:(c + 1) * P], ident[:])
                nc.scalar.activation(dst[:, c, col:col + P], tp[:],
                                     AF.Identity, bias=b1dm[:, c:c + 1],
                                     scale=g1dm[:, c:c + 1])

        # interleave: LN(query half) -> Q proj -> LN(window half) -> V -> K
        # so Q matmuls fill PE idle during LN and attention starts during K.
        for t in range(8):
            ln1_tile(t)

        qkvp = tc.alloc_tile_pool(name="qkv", bufs=1, side="right")
        qT = qkvp.tile([P, 8, WINDOW], F32R)      # q/sqrt(D), dim-major
        kT = qkvp.tile([P, 8, WINDOW], F32R)      # k, dim-major
        V = qkvp.tile([P, 8, N_HEAD * 65], F32R)  # token-major + ones col

        wst = tc.alloc_tile_pool(name="wst", bufs=1, side="left")
        psC = tc.alloc_tile_pool(name="psC", bufs=4, space="PSUM")

        # Q: weights stationary -> qT dim-major, scaled by 1/32
        for wh in range(2):
            wqr = wst.tile([P, 8, 512], F32R, tag="wkres")
            for kc in range(8):
                nc.sync.dma_start(
                    wqr[:, kc, :],
                    wq_d[kc * P:(kc + 1) * P, wh * 512:(wh + 1) * 512])
            for co in range(wh * 4, wh * 4 + 4):
                for qh in range(2):
                    pp = psC.tile([P, 512], F32, tag="proj")
                    for kc in range(8):
                        nc.tensor.matmul(
                            pp[:], wqr[:, kc, (co % 4) * P:(co % 4 + 1) * P],
                            zqT[:, kc, qh * 512:(qh + 1) * 512],
                            start=(kc == 0), stop=(kc == 7))
                    nc.scalar.activation(
                        qT[:, co, qh * 512:(qh + 1) * 512], pp[:],
                        AF.Identity, bias=bqs[:, co:co + 1], scale=ISD)

        for t in range(8, 16):
            ln1_tile(t)

        # V: activations stationary -> token-major, bias added via bcast tile
        bkvvb = wst.tile([P, D], F32, tag="bkvvb")
        nc.sync.dma_start(bkvvb[:], bkvvb_d[:])
        for vh in range(2):
            wvr = wst.tile([P, 8, 512], F32R, tag="wkres")
            for kc in range(8):
                nc.sync.dma_start(
                    wvr[:, kc, :],
                    wkv_d[kc * P:(kc + 1) * P,
                          D + vh * 512:D + (vh + 1) * 512])
            for tt in range(8):
                pp = psC.tile([P, 512], F32, tag="proj")
                for kc in range(8):
                    nc.tensor.matmul(
                        pp[:], zwT[:, kc, tt * P:(tt + 1) * P],
                        wvr[:, kc, :],
                        start=(kc == 0), stop=(kc == 7))
                vdst = V[:, tt, :].rearrange("p (h n) -> p h n", n=65)[
                    :, vh * 8:(vh + 1) * 8, 0:64]
                nc.vector.scalar_tensor_tensor(
                    vdst, pp[:].rearrange("p (h n) -> p h n", n=64), 0.0,
                    bkvvb[:, vh * 512:(vh + 1) * 512].rearrange(
                        "p (h n) -> p h n", n=64),
                    op0=ALU.add, op1=ALU.add)
        for tt in range(8):
            nc.scalar.copy(
                V[:, tt, :].rearrange("p (h n) -> p h n", n=65)[:, :, 64:65],
                onesc.rearrange("p (h n) -> p h n", n=1))
        # K: weights stationary -> kT dim-major (last so D overlaps it)
        for wh in range(2):
            wkr = wst.tile([P, 8, 512], F32R, tag="wkres")
            for kc in range(8):
                nc.sync.dma_start(
                    wkr[:, kc, :],
                    wkv_d[kc * P:(kc + 1) * P, wh * 512:(wh + 1) * 512])
            for co in range(wh * 4, wh * 4 + 4):
                for qh in range(2):
                    pp = psC.tile([P, 512], F32, tag="proj")
                    for kc in range(8):
                        nc.tensor.matmul(
                            pp[:], wkr[:, kc, (co % 4) * P:(co % 4 + 1) * P],
                            zwT[:, kc, qh * 512:(qh + 1) * 512],
                            start=(kc == 0), stop=(kc == 7))
                    nc.scalar.activation(
                        kT[:, co, qh * 512:(qh + 1) * 512], pp[:],
                        AF.Identity, bias=bkvk[:, co:co + 1], scale=1.0)

        psC.release()
        psB.release()
        wst.release()
        xz.release()
        zTp.release()

        # ---------------- Phase D: attention --------------------------------
        attnp = tc.alloc_tile_pool(name="attn", bufs=1, side="left")
        attn = attnp.tile([P, 8, D], F32)          # normalized attn out
        asum = attnp.tile([P, 8, N_HEAD], F32)     # per-head row sums
        rinva = attnp.tile([P, 8, N_HEAD], F32)    # per-head 1/rowsum

        mkp = tc.alloc_tile_pool(name="mk", bufs=1, side="left")
        sbD = tc.alloc_tile_pool(name="sbD", bufs=2, side="left")
        ssbp = tc.alloc_tile_pool(name="ssbp", bufs=4, side="left")
        ptp = tc.alloc_tile_pool(name="ptp", bufs=9, side="right")
        psDs = tc.alloc_tile_pool(name="psDs", bufs=5, space="PSUM")
        psDa = tc.alloc_tile_pool(name="psDa", bufs=2, space="PSUM")
        psDt = tc.alloc_tile_pool(name="psDt", bufs=1, space="PSUM")

        maskT = mkp.tile([P, 8, WINDOW], BF16)
        nc.sync.dma_start(maskT[:], maskT_d.rearrange("(c p) n -> p c n", p=P))
        for hp in range(N_HEAD // 2):
            pair = (2 * hp, 2 * hp + 1)
            # scores + mask + exp; consecutive matmuls alternate PE row
            # groups (partitions 0-63 vs 64-127) so LDWEIGHTS pulls ahead
            pts = {h: [] for h in pair}
            for kc in range(8):
                ssbs = {}
                for h in pair:
                    ptile = ptp.tile([P, 1024], F32R, tag="pt")
                    pts[h].append(ptile)
                    stile = ssbp.tile([P, 1024], F32, tag="ssb")
                    ssbs[h] = stile
                for qh in range(2):
                    for h in pair:
                        po, ch = (h % 2) * 64, h // 2
                        sps = psDs.tile([P, 512], F32, tag="s")
                        nc.tensor.matmul(
                            sps[:],
                            kT[po:po + 64, ch, kc * P:(kc + 1) * P],
                            qT[po:po + 64, ch, qh * 512:(qh + 1) * 512],
                            start=True, stop=True)
                        nc.vector.tensor_tensor(
                            ssbs[h][:, qh * 512:(qh + 1) * 512], sps[:],
                            maskT[:, kc, qh * 512:(qh + 1) * 512], op=ALU.min)
                for h in pair:
                    nc.scalar.activation(pts[h][kc][:], ssbs[h][:], AF.Exp)
            for h in pair:
                oa = sbD.tile([65, 1024], F32, tag="oa")
                for qh in range(2):
                    avp = psDa.tile([65, 512], F32, tag="av")
                    for kc in range(8):
                        nc.tensor.matmul(
                            avp[:], V[:, kc, h * 65:(h + 1) * 65],
                            pts[h][kc][:, qh * 512:(qh + 1) * 512],
                            start=(kc == 0), stop=(kc == 7))
                    nc.scalar.copy(oa[:, qh * 512:(qh + 1) * 512], avp[:])
                for t in range(8):
                    tp = psDt.tile([P, 65], F32, tag="tp65")
                    nc.tensor.transpose(tp[:], oa[:, t * P:(t + 1) * P],
                                        ident[0:65, 0:65])
                    rinv = rinva[:, t, h:h + 1]
                    nc.vector.reciprocal(rinv, tp[:, 64:65])
                    nc.scalar.activation(
                        attn[:, t, h * 64:(h + 1) * 64], tp[:, 0:64],
                        AF.Copy, scale=rinv, accum_out=asum[:, t, h:h + 1])

        psDt.release()
        psDa.release()
        psDs.release()
        ptp.release()
        ssbp.release()
        sbD.release()
        mkp.release()
        qkvp.release()

        # ---------------- Phase E: LN2 + transpose ---------------------------
        z2Tp = tc.alloc_tile_pool(name="z2T", bufs=1, side="right")
        z2T = z2Tp.tile([P, 8, WINDOW], F32R)
        xz2 = tc.alloc_tile_pool(name="xz2", bufs=3, side="left")
        psE = tc.alloc_tile_pool(name="psE", bufs=3, space="PSUM")

        for t in range(8):
            at = attn[:, t, :]
            st = xz2.tile([P, 8], F32, tag="stats2")
            musum, mu, vsum = st[:, 0:1], st[:, 1:2], st[:, 2:3]
            veps, sdv, rstd = st[:, 4:5], st[:, 5:6], st[:, 6:7]
            nc.vector.reduce_sum(musum, asum[:, t, :], axis=AX.X)
            nc.vector.tensor_scalar_mul(mu, musum, 1.0 / D)
            z = xz2.tile([P, D], F32, tag="zE")
            nc.vector.scalar_tensor_tensor(
                z[:], at, mu, at,
                op0=ALU.subtract, op1=ALU.mult, accum_out=vsum)
            nc.vector.tensor_scalar(veps, vsum, 1.0 / D, EPS,
                                    op0=ALU.mult, op1=ALU.add)
            nc.scalar.sqrt(sdv, veps)
            nc.vector.reciprocal(rstd, sdv)
            nc.vector.tensor_scalar(z[:], at, mu, rstd,
                                    op0=ALU.subtract, op1=ALU.mult)
            for c in range(8):
                tp = psE.tile([P, P], F32, tag="tpE")
                nc.tensor.transpose(tp[:], z[:, c * P:(c + 1) * P], ident[:])
                nc.scalar.activation(z2T[:, c, t * P:(t + 1) * P], tp[:],
                                     AF.Identity, bias=b2dm[:, c:c + 1],
                                     scale=g2dm[:, c:c + 1])

        psE.release()
        xz2.release()
        attnp.release()

        # ---------------- Phase F: MLP ---------------------------------------
        h2p = tc.alloc_tile_pool(name="h2acc", bufs=1, side="left")
        h2acc = h2p.tile([P, 8, WINDOW], F32)
        wf1 = tc.alloc_tile_pool(name="wf1", bufs=2, side="right")
        wf2 = tc.alloc_tile_pool(name="wf2", bufs=1, side="right")
        h1p = tc.alloc_tile_pool(name="h1p", bufs=1, side="left")
        psF1 = tc.alloc_tile_pool(name="psF1", bufs=3, space="PSUM")
        psF2 = tc.alloc_tile_pool(name="psF2", bufs=3, space="PSUM")

        for sc in range(4):
            w1r = wf1.tile([P, 8, 1024], F32R, tag="w1r")
            for kc in range(8):
                nc.sync.dma_start(
                    w1r[:, kc, :],
                    w1_d[kc * P:(kc + 1) * P, sc * 1024:(sc + 1) * 1024])
            h1 = h1p.tile([P, 8, WINDOW], F32R, tag="h1")
            for ft in range(8):
                for qh in range(2):
                    hp = psF1.tile([P, 512], F32, tag="h1ps")
                    for kc in range(8):
                        nc.tensor.matmul(
                            hp[:], w1r[:, kc, ft * P:(ft + 1) * P],
                            z2T[:, kc, qh * 512:(qh + 1) * 512],
                            start=(kc == 0), stop=(kc == 7))
                    nc.scalar.activation(
                        h1[:, ft, qh * 512:(qh + 1) * 512], hp[:], AF.Silu,
                        bias=b1s[:, sc * 8 + ft:sc * 8 + ft + 1], scale=1.0)
            w2r = wf2.tile([P, 8, 1024], F32R, tag="w2r")
            for kc in range(8):
                nc.sync.dma_start(
                    w2r[:, kc, :],
                    w2_d[(sc * 8 + kc) * P:(sc * 8 + kc + 1) * P, :])
            for co in range(8):
                for qh in range(2):
                    hp2 = psF2.tile([P, 512], F32, tag="h2ps")
                    for kc in range(8):
                        nc.tensor.matmul(
                            hp2[:], w2r[:, kc, co * P:(co + 1) * P],
                            h1[:, kc, qh * 512:(qh + 1) * 512],
                            start=(kc == 0), stop=(kc == 7))
                    dstp = h2acc[:, co, qh * 512:(qh + 1) * 512]
                    if sc == 0:
                        nc.vector.tensor_copy(dstp, hp2[:])
                    elif sc < 3:
                        nc.vector.tensor_tensor(dstp, hp2[:], dstp,
                                                op=ALU.add)
                    else:
                        nc.vector.scalar_tensor_tensor(
                            dstp, hp2[:], b2s[:, co:co + 1], dstp,
                            op0=ALU.add, op1=ALU.add)

        psF2.release()
        psF1.release()
        h1p.release()
        wf2.release()
        wf1.release()
        z2Tp.release()

        # tail: residual add in dim-major (host supplies x^T and
        # transposes y back), no PE transposes needed
        tailp = tc.alloc_tile_pool(name="tail", bufs=3, side="left")
        xinTp = tc.alloc_tile_pool(name="xinT", bufs=1, side="left")
        xinT = xinTp.tile([P, 8, WINDOW], F32)
        nc.sync.dma_start(xinT[:], xinT_d.rearrange("(c p) n -> p c n", p=P))
        for co in range(8):
            y = tailp.tile([P, WINDOW], F32, tag="y")
            nc.vector.tensor_tensor(y[:], h2acc[:, co, :], xinT[:, co, :],
                                    op=ALU.add)
            nc.sync.dma_start(y_d[co * P:(co + 1) * P, :], y[:])
        xinTp.release()
        tailp.release()
        h2p.release()
        cpool.release()

    nc.compile()
    return nc


def _prep_inputs(inputs):
    x = np.ascontiguousarray(np.asarray(inputs["x"], dtype=np.float32))
    kpm = np.asarray(inputs["key_pad_mask"]).astype(bool)
    wq = np.ascontiguousarray(np.asarray(inputs["wq"], dtype=np.float32))
    wkv = np.ascontiguousarray(np.asarray(inputs["wkv"], dtype=np.float32))
    w1 = np.ascontiguousarray(np.asarray(inputs["w1"], dtype=np.float32))
    w2 = np.ascontiguousarray(np.asarray(inputs["w2"], dtype=np.float32))
    bq = np.asarray(inputs["bq"], dtype=np.float32)
    bkv = np.asarray(inputs["bkv"], dtype=np.float32)
    b1 = np.asarray(inputs["b1"], dtype=np.float32)
    b2 = np.asarray(inputs["b2"], dtype=np.float32)
    ln1_g = np.asarray(inputs["ln1_g"], dtype=np.float32)
    ln1_b = np.asarray(inputs["ln1_b"], dtype=np.float32)
    ln2_g = np.asarray(inputs["ln2_g"], dtype=np.float32)
    ln2_b = np.asarray(inputs["ln2_b"], dtype=np.float32)

    def dm(v):  # [D] -> [P, 8] dim-major chunk layout
        return np.ascontiguousarray(v.reshape(8, P).T)

    shared = {
        "wq": wq,
        "wkv": wkv,
        "w1": w1,
        "w2": w2,
        "bqs": np.ascontiguousarray((bq * ISD).reshape(8, P).T),
        "bkvk": dm(bkv[0:D]),
        "bkvvb": np.ascontiguousarray(
            np.broadcast_to(bkv[D:2 * D], (P, D)).astype(np.float32)),
        "g1dm": dm(ln1_g),
        "b1dm": dm(ln1_b),
        "g2dm": dm(ln2_g),
        "b2dm": dm(ln2_b),
        "b1s": np.ascontiguousarray(b1.reshape(32, P).T),
        "b2s": dm(b2),
    }

    j = np.arange(WINDOW)[:, None]   # key index within window (row)
    i = np.arange(WINDOW)[None, :]   # local query index (col)
    in_maps = []
    for core in range(8):
        b, h = core // 2, core % 2
        xq = x[b, h * WINDOW:(h + 1) * WINDOW]
        xw = x[b, S - WINDOW:S]
        masked = (j > h * WINDOW + i) | kpm[b, S - WINDOW:S][:, None]
        maskT = np.where(masked, np.float32(MASKVAL),
                         np.float32(KEEPVAL)).astype(ml_dtypes.bfloat16)
        m = dict(shared)
        m["xin"] = np.ascontiguousarray(np.concatenate([xq, xw], axis=0))
        m["xinT"] = np.ascontiguousarray(xq.T)
        m["maskT"] = np.ascontiguousarray(maskT)
        in_maps.append(m)
    return in_maps


def kernel(**inputs):
    from concourse.bass_utils import run_bass_kernel_spmd

    if "nc" not in _CACHE:
        _CACHE["nc"] = _build_program()
    nc = _CACHE["nc"]

    in_maps = _prep_inputs(inputs)
    trace = os.environ.get("KERNEL_TRACE", "0") == "1"
    res = run_bass_kernel_spmd(nc, in_maps, core_ids=list(range(8)),
                               trace=trace)
    if res.exec_time_ns is not None:
        print(f"HW exec time: {res.exec_time_ns} ns")
        _CACHE["exec_time_ns"] = res.exec_time_ns
    out = np.empty((B, S, D), dtype=np.float32)
    for core in range(8):
        b, h = core // 2, core % 2
        out[b, h * WINDOW:(h + 1) * WINDOW] = res.results[core]["y"].T
    return out
